# revision 1
# baseline (speedup 1.0000x reference)
"""Trainium2 Bass kernel for batched multi-head attention (8 NeuronCores).

Problem: x[8,1024,1024], Wq[1024,1024], bq[1024], Wkv[1024,2048], bkv[2048]
  q = x@Wq+bq ; k,v = split(x@Wkv+bkv, 2) ;
  out = softmax(q k^T / sqrt(64), over keys) @ v        (16 heads, d=64)

Sharding: data-parallel over batch - one batch element per NeuronCore;
weights replicated; no collectives. Outputs are stacked on the host.

Per-core kernel design (bf16 matmuls, fp32 PSUM):
  - x^T via transposing DMA; q^T/k^T (c on partitions) and v (tokens on
    partitions) on the PE; scores s^T = k^T.T @ q^T put softmax keys on
    the partition dim so attention probabilities come out pre-transposed
    for the att@v matmul.
  - The k-projection bias is dropped entirely: softmax over keys is
    invariant to the per-query constant q.bk it adds to every logit.
  - Score matmuls for head pair (2cc, 2cc+1) issue as two concurrent
    row-group matmuls (tile_position (0,0)/(64,0)), using that the pair's
    d-rows live at partitions 0-63/64-127 of kT/qT[cc]. Measured 170ns/MM
    vs 483ns for the naive K=64 form, where LDWEIGHTS cannot overlap an
    in-flight matmul on the same row group.
  - exp on ScalarE, one instruction per [128, 2048] (two score slots):
    ~2.1us each vs 2x1.6us at [128,1024] - ScalarE drops 201->137us and
    stays off the critical path.
  - v carries a ones-column, so att@v also emits the softmax denominator;
    one batched reciprocal per head ([128,8] strided from PSUM), then a
    fused (o*rz + bv) VectorE op per query block.
  - Emission interleaves PE work explicitly (Tile executes each engine's
    queue in program order): between score slots, PE drains att@v groups
    of the previous pair and projection half-groups of the next pair, so
    it never stalls on ScalarE.
  - Output staged bf16 (error budget allows 0.4%), upcast on host.
"""

from contextlib import ExitStack

import numpy as np
import ml_dtypes

import concourse.bass as bass
import concourse.mybir as mybir
import concourse.tile as tile
from concourse import bacc
from concourse.bass_utils import run_bass_kernel_spmd

P = 128
N = 1024
C = 1024
H = 16
D = 64
NCH = N // P
B = 8
SCALE = D ** -0.5
F32 = mybir.dt.float32
BF16 = mybir.dt.bfloat16
EXP = mybir.ActivationFunctionType.Exp
MULT = mybir.AluOpType.mult
ADD = mybir.AluOpType.add


def build(repeats=1):
    nc = bacc.Bacc("TRN2")
    x = nc.dram_tensor("x", [N, C], BF16, kind="ExternalInput")
    wq = nc.dram_tensor("wq", [C, C], BF16, kind="ExternalInput")
    bq = nc.dram_tensor("bq", [C], F32, kind="ExternalInput")
    wkv = nc.dram_tensor("wkv", [C, 2 * C], BF16, kind="ExternalInput")
    bkv = nc.dram_tensor("bkv", [2 * C], F32, kind="ExternalInput")
    out = nc.dram_tensor("out", [N, C], BF16, kind="ExternalOutput")

    with ExitStack() as ctx:
        tc = ctx.enter_context(tile.TileContext(nc))
        persist = ctx.enter_context(tc.tile_pool(name="persist", bufs=1))

        xT_t = [persist.tile([P, N], BF16, tag=f"xT{i}", name=f"xT{i}")
                for i in range(NCH)]
        wq_t = [persist.tile([P, C], BF16, tag=f"wq{i}", name=f"wq{i}")
                for i in range(NCH)]
        wkv_t = [persist.tile([P, 2 * C], BF16, tag=f"wkv{i}", name=f"wkv{i}")
                 for i in range(NCH)]
        qT_t = [persist.tile([P, N], BF16, tag=f"qT{i}", name=f"qT{i}")
                for i in range(NCH)]
        kT_t = [persist.tile([P, N], BF16, tag=f"kT{i}", name=f"kT{i}")
                for i in range(NCH)]
        v_sb = persist.tile([P, NCH, H, D + 1], BF16, tag="v")
        out_t = [persist.tile([P, C], BF16, tag=f"out{i}", name=f"out{i}")
                 for i in range(NCH)]
        bq_sb = persist.tile([P, NCH], F32, tag="bq")
        bv_bc = persist.tile([P, C], F32, tag="bv")
        scratch = persist.tile([P, 512], BF16, tag="scratch")

        # pt: both heads of one pair; [P, mc, nh, hh, 512]
        pt_pool = ctx.enter_context(tc.tile_pool(name="pt", bufs=2))
        rz_pool = ctx.enter_context(tc.tile_pool(name="rz", bufs=2))
        proj_ps = ctx.enter_context(
            tc.tile_pool(name="proj_ps", bufs=2, space="PSUM"))
        s_ps = ctx.enter_context(tc.tile_pool(name="s_ps", bufs=1, space="PSUM"))
        o_ps = ctx.enter_context(tc.tile_pool(name="o_ps", bufs=1, space="PSUM"))

        nc.vector.memset(scratch[:], 0.0)
        nc.vector.memset(v_sb[:], 1.0)
        wps = proj_ps.tile([P, 512], F32, tag="ps", name="wups")
        for _ in range(10):
            nc.tensor.matmul(wps[:], scratch[:, 0:P], scratch[:],
                             start=True, stop=True)

        for rep in range(repeats):
            for cc in range(NCH):
                nc.sync.dma_start(out=xT_t[cc][:],
                                  in_=x.ap()[:, cc * P:(cc + 1) * P],
                                  transpose=True)
            for kc in range(NCH):
                nc.sync.dma_start(out=wkv_t[kc][:],
                                  in_=wkv.ap()[kc * P:(kc + 1) * P, :])
            for kc in range(NCH):
                nc.scalar.dma_start(out=wq_t[kc][:],
                                    in_=wq.ap()[kc * P:(kc + 1) * P, :])
            nc.sync.dma_start(out=bq_sb[:],
                              in_=bq.ap().rearrange("(cc p) -> p cc", p=P))
            bv_row = bkv.ap()[C:2 * C]
            nc.sync.dma_start(
                out=bv_bc[:],
                in_=bass.AP(tensor=bv_row.tensor, offset=bv_row.offset,
                            ap=[[0, P]] + list(bv_row.ap)),
            )

            # ---- chunk builders ----

            def mk_dual_proj(dst, lhs_fn, half, bias=None):
                # one half (4 kc chunks) of BOTH nh groups, interleaved so
                # the two matmuls per kc share one stationary back-to-back
                def emit():
                    if half == 0:
                        psa = proj_ps.tile([P, 512], F32, tag="ps", name="psa")
                        psb = proj_ps.tile([P, 512], F32, tag="ps", name="psb")
                        emit.ps = (psa, psb)
                    else:
                        psa, psb = emit.partner.ps
                    for kc in range(4 * half, 4 * half + 4):
                        lhs = lhs_fn(kc)
                        nc.tensor.matmul(psa[:], lhs, xT_t[kc][:, 0:512],
                                         start=(kc == 0), stop=(kc == NCH - 1))
                        nc.tensor.matmul(psb[:], lhs, xT_t[kc][:, 512:1024],
                                         start=(kc == 0), stop=(kc == NCH - 1))
                    if half == 1:
                        for nh, ps in ((0, psa), (1, psb)):
                            sl = (slice(None),
                                  slice(nh * 512, nh * 512 + 512))
                            if bias is None:
                                nc.vector.tensor_copy(dst[sl], ps[:])
                            else:
                                nc.vector.tensor_scalar_add(dst[sl], ps[:],
                                                            bias)
                return emit

            def proj_chunks(cc2):
                chunks = []
                for kind in ("k", "q"):
                    if kind == "k":
                        lhs = (lambda kc, cc2=cc2:
                               wkv_t[kc][:, cc2 * P:(cc2 + 1) * P])
                        dst, bias = kT_t[cc2], None
                    else:
                        lhs = (lambda kc, cc2=cc2:
                               wq_t[kc][:, cc2 * P:(cc2 + 1) * P])
                        dst, bias = qT_t[cc2], bq_sb[:, cc2:cc2 + 1]
                    h0 = mk_dual_proj(dst, lhs, 0)
                    h1 = mk_dual_proj(dst, lhs, 1, bias=bias)
                    h1.partner = h0
                    chunks += [h0, h1]
                return chunks

            def mk_v_proj(mc, half):
                # both ch groups for query-block mc, stationary shared per kc
                def emit():
                    if half == 0:
                        psa = proj_ps.tile([P, 512], F32, tag="ps", name="psa")
                        psb = proj_ps.tile([P, 512], F32, tag="ps", name="psb")
                        emit.ps = (psa, psb)
                    else:
                        psa, psb = emit.partner.ps
                    for kc in range(4 * half, 4 * half + 4):
                        lhs = xT_t[kc][:, mc * P:(mc + 1) * P]
                        nc.tensor.matmul(psa[:], lhs,
                                         wkv_t[kc][:, C:C + 512],
                                         start=(kc == 0), stop=(kc == NCH - 1))
                        nc.tensor.matmul(psb[:], lhs,
                                         wkv_t[kc][:, C + 512:C + 1024],
                                         start=(kc == 0), stop=(kc == NCH - 1))
                    if half == 1:
                        for ch, ps in ((0, psa), (1, psb)):
                            nc.vector.tensor_copy(
                                v_sb[:, mc, ch * 8:(ch + 1) * 8, 0:D],
                                ps[:].rearrange("p (h d) -> p h d", d=D))
                return emit

            def mk_attv_group(pt, hh, h, ni, o_ref):
                def emit():
                    if ni == 0:
                        o_ref[0] = o_ps.tile([P, NCH, P], F32,
                                             tag="o", name="o")
                    o_all = o_ref[0]
                    nh, col = ni // 4, (ni % 4) * P
                    for mc in range(NCH):
                        nc.tensor.matmul(
                            o_all[:, ni, 0:D + 1],
                            pt[:, mc, nh, hh, col:col + P],
                            v_sb[:, mc, h, :],
                            start=(mc == 0), stop=(mc == NCH - 1))
                return emit

            def mk_norm(h, o_ref):
                def emit():
                    o_all = o_ref[0]
                    rz = rz_pool.tile([P, NCH], F32, tag="rz", name="rz")
                    nc.vector.reciprocal(rz[:], o_all[:, :, D])
                    for ni in range(NCH):
                        nc.vector.scalar_tensor_tensor(
                            out_t[ni][:, h * D:(h + 1) * D],
                            o_all[:, ni, 0:D], rz[:, ni:ni + 1],
                            bv_bc[:, h * D:(h + 1) * D], op0=MULT, op1=ADD)
                return emit

            # ---- interleaved emission ----
            highq = []
            lowq = []

            def drain(q, n):
                for _ in range(min(n, len(q))):
                    q.pop(0)()

            for chunk in proj_chunks(0):
                chunk()

            for cc in range(NCH):
                h0, h1 = 2 * cc, 2 * cc + 1
                pt = pt_pool.tile([P, NCH, 2, 2, 512], BF16, tag="pt",
                                  name="pt")
                if cc == 0:
                    for mc in range(NCH):
                        a = mk_v_proj(mc, 0)
                        b = mk_v_proj(mc, 1)
                        b.partner = a
                        lowq += [a, b]
                if cc + 1 < NCH:
                    highq.extend(proj_chunks(cc + 1))

                for mc in range(NCH):
                    s2 = s_ps.tile([P, 2, 2, 512], F32, tag="s", name="s")
                    for nh in range(2):
                        nc.tensor.matmul(
                            s2[:, nh, 0, :],
                            kT_t[cc][0:D, mc * P:(mc + 1) * P],
                            qT_t[cc][0:D, nh * 512:(nh + 1) * 512],
                            start=True, stop=True, tile_position=(0, 0))
                        nc.tensor.matmul(
                            s2[:, nh, 1, :],
                            kT_t[cc][D:2 * D, mc * P:(mc + 1) * P],
                            qT_t[cc][D:2 * D, nh * 512:(nh + 1) * 512],
                            start=True, stop=True, tile_position=(D, 0))
                    nc.scalar.activation(pt[:, mc, :, :, :], s2[:],
                                         EXP, scale=SCALE)
                    drain(highq, 1)
                    drain(lowq, 3)

                o0, o1 = [None], [None]
                lowq += [mk_attv_group(pt, 0, h0, ni, o0) for ni in range(NCH)]
                lowq += [mk_norm(h0, o0)]
                lowq += [mk_attv_group(pt, 1, h1, ni, o1) for ni in range(NCH)]
                lowq += [mk_norm(h1, o1)]

            while highq or lowq:
                drain(highq, 99)
                drain(lowq, 99)

            for ni in range(NCH):
                eng = nc.sync if ni % 2 == 0 else nc.scalar
                eng.dma_start(out=out.ap()[ni * P:(ni + 1) * P, :],
                              in_=out_t[ni][:])

    nc.finalize()
    return nc


def make_in_maps(inputs):
    bf = ml_dtypes.bfloat16
    x = np.asarray(inputs["x"])
    wq_b = np.asarray(inputs["Wq"]).astype(bf)
    wkv_b = np.asarray(inputs["Wkv"]).astype(bf)
    bq_f = np.asarray(inputs["bq"]).astype(np.float32)
    bkv_f = np.asarray(inputs["bkv"]).astype(np.float32)
    return [
        {"x": x[b].astype(bf), "wq": wq_b, "bq": bq_f, "wkv": wkv_b,
         "bkv": bkv_f}
        for b in range(B)
    ]


_NC = None


def kernel(x, Wq, bq, Wkv, bkv):
    global _NC
    if _NC is None:
        _NC = build()
    in_maps = make_in_maps(
        {"x": x, "Wq": Wq, "bq": bq, "Wkv": Wkv, "bkv": bkv})
    res = run_bass_kernel_spmd(_NC, in_maps, core_ids=list(range(B)))
    return np.stack([res.results[b]["out"] for b in range(B)]).astype(
        np.float32)

